# revision 8
# baseline (speedup 1.0000x reference)
"""ViT-Base/16 forward pass on 8 TRN2 NeuronCores (data-parallel over batch).

Layout: feature-major activations [768, T=788] per core (4 images x 197 tokens),
stored as 6 chunks of [128, T]. GEMMs run in float32r (full PE rate, even-N
requirement -> token halves of 394). Attention internals run in bf16.
LayerNorm affine params are folded into the following GEMM weights host-side.
"""
import os
import numpy as np

import concourse.bass as bass
import concourse.bacc as bacc
import concourse.tile as tile
from concourse import mybir
from concourse.bass_utils import run_bass_kernel_spmd
from concourse.masks import make_identity

F32 = mybir.dt.float32
F32R = mybir.dt.float32r
BF16 = mybir.dt.bfloat16
AF = mybir.ActivationFunctionType
OP = mybir.AluOpType

D, DEPTH, H, DK, MLP = 768, 12, 12, 64, 3072
NIMG = 4            # images per core
NTOK = 197          # tokens per image
T = NIMG * NTOK     # 788
HALF = T // 2       # 394
KC = D // 128       # 6
MC = MLP // 128     # 24
SCALE = DK ** -0.5  # 0.125
EPS = 1e-5
NCORES = 8

_NC_CACHE = {}


def _build_nc():
    nc = bacc.Bacc()
    dp = nc.declare_dram_parameter

    patches = dp("patches", [KC, 128, NIMG * 196], F32, isOutput=False)
    clspos = dp("clspos", [KC, 128], F32, isOutput=False)
    pos = dp("pos", [KC, 128, NTOK], F32, isOutput=False)
    convw = dp("convw", [KC, 128, D], F32, isOutput=False)
    convb = dp("convb", [KC, 128], F32, isOutput=False)
    qkvw = dp("qkvw", [DEPTH, KC, 128, 3 * D], F32, isOutput=False)
    qkvb = dp("qkvb", [DEPTH, 3 * KC, 128], F32, isOutput=False)
    wow = dp("wow", [DEPTH, KC, 128, D], F32, isOutput=False)
    wob = dp("wob", [DEPTH, KC, 128], F32, isOutput=False)
    m1w = dp("m1w", [DEPTH, KC, 128, MLP], F32, isOutput=False)
    m1b = dp("m1b", [DEPTH, MC, 128], F32, isOutput=False)
    m2w = dp("m2w", [DEPTH, MC, 128, D], F32, isOutput=False)
    m2b = dp("m2b", [DEPTH, KC, 128], F32, isOutput=False)
    fnw = dp("fnw", [KC, 128], F32, isOutput=False)
    fnb = dp("fnb", [KC, 128], F32, isOutput=False)
    h1w = dp("h1w", [KC, 128, MLP], F32, isOutput=False)
    h1b = dp("h1b", [MC, 128], F32, isOutput=False)
    h2w = dp("h2w", [MC, 128, 2], F32, isOutput=False)
    h2b = dp("h2b", [2, 1], F32, isOutput=False)
    out = dp("out", [NIMG, 2], F32, isOutput=True)

    with tile.TileContext(nc) as tc:
        with (
            tc.tile_pool(name="persist", bufs=1) as pp,
            tc.tile_pool(name="work", bufs=1) as wk,
            tc.tile_pool(name="wpool", bufs=1) as wp,
            tc.tile_pool(name="ps", bufs=1, space="PSUM") as ps,
        ):
            # ---- constants ----
            ident = pp.tile([128, 128], BF16)
            make_identity(nc, ident)
            ones_bf = pp.tile([128, 1], BF16)
            nc.vector.memset(ones_bf, 1.0)
            eps_t = pp.tile([1, 1], F32)
            nc.vector.memset(eps_t, EPS)

            pos_t = [pp.tile([128, NTOK], F32, tag=f"pos{c}", name=f"pos{c}") for c in range(KC)]
            for c in range(KC):
                nc.sync.dma_start(out=pos_t[c], in_=pos[c, :, :])

            # persistent activations
            tok = [pp.tile([128, T], F32, tag=f"tok{c}", name=f"tok{c}") for c in range(KC)]
            q_bf = [pp.tile([128, T], BF16, tag=f"q{c}", name=f"qb{c}") for c in range(KC)]
            k_bf = [pp.tile([128, T], BF16, tag=f"k{c}", name=f"kb{c}") for c in range(KC)]
            v_bf = [pp.tile([128, T], BF16, tag=f"v{c}", name=f"vb{c}") for c in range(KC)]
            ctx = [pp.tile([128, T], F32R, tag=f"ctx{c}", name=f"ctx{c}") for c in range(KC)]
            hmlp = pp.tile([128, MC, HALF], F32R, tag="hmlp")

            def gemm_smallk(w_src, n_dout, rhs_fn, consumer, nhalf=HALF,
                            G=4, kcn=KC):
                """w_src(kc)->dram AP [128, n_dout*128]; rhs_fn(kc,half)->sbuf
                [128,nhalf] f32r; consumer(dc, half, psum)."""
                for g0 in range(0, n_dout, G):
                    gsz = min(G, n_dout - g0)
                    slabs = []
                    for kc in range(kcn):
                        wt = wp.tile([128, gsz * 128], F32R, tag="wslab", bufs=8)
                        nc.sync.dma_start(
                            out=wt,
                            in_=w_src(kc)[:, g0 * 128:(g0 + gsz) * 128].bitcast(F32R))
                        slabs.append(wt)
                    for half in range(2):
                        for dc in range(gsz):
                            p = ps.tile([128, nhalf], F32, tag="gemm", bufs=4)
                            for kc in range(kcn):
                                nc.tensor.matmul(
                                    p, slabs[kc][:, dc * 128:(dc + 1) * 128],
                                    rhs_fn(kc, half),
                                    start=(kc == 0), stop=(kc == kcn - 1))
                            consumer(g0 + dc, half, p)

            def layernorm(src_slice_fn, dst, half_w=HALF, nchunk=KC,
                          scale_fn=None):
                """Feature-major LN over partitions(x chunks). src_slice_fn(kc,
                half)->[128,half_w] fp32 AP. dst[kc] f32r tiles [128, KC, half_w]
                indexed [:, kc, :] per half -> writes dst(kc, half, ap)."""
                inv = 1.0 / (128 * nchunk)
                for half in range(2):
                    xb, xs = [], []
                    for kc in range(nchunk):
                        b = wk.tile([128, half_w], BF16, tag="lnxb", bufs=6)
                        nc.gpsimd.tensor_copy(out=b, in_=src_slice_fn(kc, half))
                        s = wk.tile([128, half_w], BF16, tag="lnxs", bufs=6)
                        nc.scalar.activation(out=s, in_=src_slice_fn(kc, half),
                                             func=AF.Square)
                        xb.append(b)
                        xs.append(s)
                    psum = ps.tile([1, half_w], F32, tag="attps", bufs=4)
                    psq = ps.tile([1, half_w], F32, tag="attps", bufs=4)
                    for kc in range(nchunk):
                        nc.tensor.matmul(psum, ones_bf, xb[kc],
                                         start=(kc == 0), stop=(kc == nchunk - 1))
                    for kc in range(nchunk):
                        nc.tensor.matmul(psq, ones_bf, xs[kc],
                                         start=(kc == 0), stop=(kc == nchunk - 1))
                    st = wk.tile([1, 2, half_w], F32, tag="lnst", bufs=2)
                    # mean
                    nc.scalar.mul(out=st[:, 0, :], in_=psum[:, :], mul=inv)
                    # m2 = mean^2
                    m2 = wk.tile([1, half_w], F32, tag="lnm2", bufs=2)
                    nc.vector.tensor_mul(m2, st[:, 0, :], st[:, 0, :])
                    # var = psq*inv - m2
                    var = wk.tile([1, half_w], F32, tag="lnvar", bufs=2)
                    nc.vector.scalar_tensor_tensor(
                        out=var, in0=psq[:, :], scalar=inv, in1=m2,
                        op0=OP.mult, op1=OP.subtract)
                    # rstd = 1/sqrt(var + eps)
                    nc.scalar.activation(out=st[:, 1, :], in_=var,
                                         func=AF.Sqrt, bias=eps_t)
                    nc.vector.reciprocal(st[:, 1, :], st[:, 1, :])
                    bc = wk.tile([128, 2, half_w], F32, tag="lnbc", bufs=2)
                    nc.gpsimd.partition_broadcast(bc, st[0:1])
                    for kc in range(nchunk):
                        tmp = wk.tile([128, half_w], F32, tag="lntmp", bufs=3)
                        nc.vector.tensor_sub(tmp, src_slice_fn(kc, half),
                                             bc[:, 0, :])
                        dst(kc, half, tmp, bc)

            # ================= patch embed =================
            PW = 392  # half width of 784 patch cols
            with tc.tile_pool(name="patp", bufs=1) as patp:
                pat = [patp.tile([128, NIMG * 196], F32R, tag=f"pat{c}",
                                 name=f"pat{c}") for c in range(KC)]
                for c in range(KC):
                    nc.sync.dma_start(out=pat[c], in_=patches[c].bitcast(F32R))
                convb_t = wk.tile([128, KC], F32, tag="convb")
                nc.sync.dma_start(out=convb_t, in_=convb.rearrange("c p -> p c"))

                def conv_consumer(dc, half, p):
                    # psum cols: [0:196]/[196:392] -> img 2*half / 2*half+1
                    for j in range(2):
                        img = 2 * half + j
                        nc.scalar.activation(
                            out=tok[dc][:, img * NTOK + 1:(img + 1) * NTOK],
                            in_=p[:, j * 196:(j + 1) * 196],
                            func=AF.Gelu, bias=convb_t[:, dc:dc + 1])

                gemm_smallk(lambda kc: convw[kc], KC,
                            lambda kc, half: pat[kc][:, half * PW:(half + 1) * PW],
                            conv_consumer, nhalf=PW)

            # cls columns + positional embedding
            for c in range(KC):
                tokap = tok[c][:, :]
                dst = bass.AP(tensor=tokap.tensor, offset=tokap.offset,
                              ap=[tokap.ap[0], [NTOK, NIMG]])
                cpap = clspos[c, :]
                src = bass.AP(tensor=cpap.tensor, offset=cpap.offset,
                              ap=[cpap.ap[0], [0, NIMG]])
                nc.sync.dma_start(out=dst, in_=src)
            for c in range(KC):
                for img in range(NIMG):
                    nc.vector.tensor_add(
                        tok[c][:, img * NTOK + 1:(img + 1) * NTOK],
                        tok[c][:, img * NTOK + 1:(img + 1) * NTOK],
                        pos_t[c][:, 1:NTOK])

            # ================= transformer layers =================
            for layer in range(DEPTH):
                # ---- LN1 -> xhat (f32r) ----
                xhat = wk.tile([128, KC, 2, HALF], F32R, tag="xhat", bufs=1)

                def ln1_dst(kc, half, tmp, bc):
                    nc.vector.tensor_mul(xhat[:, kc, half, :], tmp, bc[:, 1, :])

                layernorm(lambda kc, half:
                          tok[kc][:, half * HALF:(half + 1) * HALF],
                          ln1_dst)

                # ---- QKV ----
                qkvb_t = wk.tile([128, 3 * KC], F32, tag="qkvb", bufs=2)
                nc.sync.dma_start(out=qkvb_t,
                                  in_=qkvb[layer].rearrange("c p -> p c"))

                def qkv_consumer(dc, half, p):
                    dst = (q_bf, k_bf, v_bf)[dc // KC][dc % KC]
                    nc.vector.tensor_scalar_add(
                        out=dst[:, half * HALF:(half + 1) * HALF],
                        in0=p, scalar1=qkvb_t[:, dc:dc + 1])

                gemm_smallk(lambda kc: qkvw[layer, kc], 3 * KC,
                            lambda kc, half: xhat[:, kc, half, :],
                            qkv_consumer)

                # ---- attention (per image, per head; bf16) ----
                NSPLIT = ((0, 128), (128, NTOK))
                for img in range(NIMG):
                    c0 = img * NTOK
                    for h in range(H):
                        ch, r0 = h // 2, (h % 2) * DK
                        qs = q_bf[ch][r0:r0 + DK, c0:c0 + NTOK]
                        ks = k_bf[ch][r0:r0 + DK, c0:c0 + NTOK]
                        vs = v_bf[ch][r0:r0 + DK, c0:c0 + NTOK]
                        attn = wk.tile([128, 2, NTOK], BF16, tag="attn", bufs=2)
                        for ni, (n0, n1) in enumerate(NSPLIT):
                            nsz = n1 - n0
                            sc = ps.tile([128, NTOK], F32, tag="attps", bufs=4)
                            nc.tensor.matmul(sc[0:nsz], qs[:, n0:n1], ks,
                                             start=True, stop=True)
                            ssum = wk.tile([128, 1], F32, tag="ssum", bufs=4)
                            nc.scalar.activation(
                                out=attn[0:nsz, ni, :], in_=sc[0:nsz],
                                func=AF.Exp, scale=SCALE,
                                accum_out=ssum[0:nsz])
                            nc.vector.reciprocal(ssum[0:nsz], ssum[0:nsz])
                            nc.vector.tensor_scalar_mul(
                                out=attn[0:nsz, ni, :], in0=attn[0:nsz, ni, :],
                                scalar1=ssum[0:nsz])
                        # transpose attn -> attnT[m_chunk][*, n]; V -> vT
                        attnT = [wk.tile([128, NTOK], BF16, tag=f"attnT{mi}",
                                         bufs=2, name=f"attnT{mi}") for mi in range(2)]
                        vT = [wk.tile([128, DK], BF16, tag=f"vT{mi}", bufs=2,
                                       name=f"vT{mi}")
                              for mi in range(2)]
                        for mi, (m0, m1) in enumerate(NSPLIT):
                            msz = m1 - m0
                            for ni, (n0, n1) in enumerate(NSPLIT):
                                nsz = n1 - n0
                                pt = ps.tile([128, 128], BF16, tag="attps",
                                             bufs=4)
                                nc.tensor.transpose(
                                    pt[0:msz, 0:nsz],
                                    attn[0:nsz, ni, m0:m1],
                                    ident[0:nsz, 0:nsz])
                                nc.vector.tensor_copy(
                                    out=attnT[mi][0:msz, n0:n1],
                                    in_=pt[0:msz, 0:nsz])
                            pv = ps.tile([128, DK], BF16, tag="attps", bufs=4)
                            nc.tensor.transpose(
                                pv[0:msz, :], vs[:, m0:m1],
                                ident[r0:r0 + DK, r0:r0 + DK])
                            nc.vector.tensor_copy(out=vT[mi][0:msz, :],
                                                  in_=pv[0:msz, :])
                        pc = ps.tile([64, NTOK], F32, tag="attps", bufs=4)
                        for mi, (m0, m1) in enumerate(NSPLIT):
                            msz = m1 - m0
                            nc.tensor.matmul(pc, vT[mi][0:msz, :],
                                             attnT[mi][0:msz, :],
                                             start=(mi == 0), stop=(mi == 1))
                        nc.vector.tensor_copy(
                            out=ctx[ch][r0:r0 + DK, c0:c0 + NTOK], in_=pc)

                # ---- O projection + residual ----
                wob_t = wk.tile([128, KC], F32, tag="wob", bufs=2)
                nc.sync.dma_start(out=wob_t,
                                  in_=wob[layer].rearrange("c p -> p c"))

                def wo_consumer(dc, half, p):
                    sl = tok[dc][:, half * HALF:(half + 1) * HALF]
                    nc.vector.scalar_tensor_tensor(
                        out=sl, in0=p, scalar=wob_t[:, dc:dc + 1], in1=sl,
                        op0=OP.add, op1=OP.add)

                gemm_smallk(lambda kc: wow[layer, kc], KC,
                            lambda kc, half:
                            ctx[kc][:, half * HALF:(half + 1) * HALF],
                            wo_consumer)

                # ---- LN2 -> xhat2 ----
                xhat2 = wk.tile([128, KC, 2, HALF], F32R, tag="xhat", bufs=1)

                def ln2_dst(kc, half, tmp, bc):
                    nc.vector.tensor_mul(xhat2[:, kc, half, :], tmp, bc[:, 1, :])

                layernorm(lambda kc, half:
                          tok[kc][:, half * HALF:(half + 1) * HALF],
                          ln2_dst)

                # ---- MLP1 + GELU ----
                m1b_t = wk.tile([128, MC], F32, tag="m1b", bufs=2)
                nc.sync.dma_start(out=m1b_t,
                                  in_=m1b[layer].rearrange("c p -> p c"))

                # hmlp holds one half at a time; loop halves outermost
                m2b_t = wk.tile([128, KC], F32, tag="m2b", bufs=2)
                nc.sync.dma_start(out=m2b_t,
                                  in_=m2b[layer].rearrange("c p -> p c"))

                for half in range(2):
                    def m1_cons_h(dc, _h, p, half=half):
                        nc.scalar.activation(out=hmlp[:, dc, :], in_=p,
                                             func=AF.Gelu,
                                             bias=m1b_t[:, dc:dc + 1])

                    def m1_rhs(kc, _h, half=half):
                        return xhat2[:, kc, half, :]

                    # restrict gemm helper to a single "half" by passing
                    # a one-half wrapper: run groups with halves=1
                    for g0 in range(0, MC, 4):
                        gsz = min(4, MC - g0)
                        slabs = []
                        for kc in range(KC):
                            wt = wp.tile([128, gsz * 128], F32R, tag="wslab",
                                         bufs=8)
                            nc.sync.dma_start(
                                out=wt, in_=m1w[layer, kc][:, g0 * 128:
                                                           (g0 + gsz) * 128]
                                .bitcast(F32R))
                            slabs.append(wt)
                        for dc in range(gsz):
                            p = ps.tile([128, HALF], F32, tag="gemm", bufs=4)
                            for kc in range(KC):
                                nc.tensor.matmul(
                                    p, slabs[kc][:, dc * 128:(dc + 1) * 128],
                                    xhat2[:, kc, half, :],
                                    start=(kc == 0), stop=(kc == KC - 1))
                            m1_cons_h(g0 + dc, 0, p)

                    # ---- MLP2 + residual (kc-outer over 24 chunks) ----
                    for g0 in range(0, KC, 2):
                        psums = [ps.tile([128, HALF], F32, tag="gemm", bufs=4,
                                          name="m2ps")
                                 for _ in range(2)]
                        for kc in range(MC):
                            wt = wp.tile([128, 2 * 128], F32R, tag="wslab",
                                         bufs=8)
                            nc.sync.dma_start(
                                out=wt, in_=m2w[layer, kc][:, g0 * 128:
                                                           (g0 + 2) * 128]
                                .bitcast(F32R))
                            for dc in range(2):
                                nc.tensor.matmul(
                                    psums[dc], wt[:, dc * 128:(dc + 1) * 128],
                                    hmlp[:, kc, :],
                                    start=(kc == 0), stop=(kc == MC - 1))
                        for dc in range(2):
                            sl = tok[g0 + dc][:, half * HALF:(half + 1) * HALF]
                            nc.vector.scalar_tensor_tensor(
                                out=sl, in0=psums[dc],
                                scalar=m2b_t[:, g0 + dc:g0 + dc + 1], in1=sl,
                                op0=OP.add, op1=OP.add)

            # ================= final head (cls tokens only) =================
            fnw_t = wk.tile([128, KC], F32, tag="fnw")
            fnb_t = wk.tile([128, KC], F32, tag="fnb")
            nc.sync.dma_start(out=fnw_t, in_=fnw.rearrange("c p -> p c"))
            nc.sync.dma_start(out=fnb_t, in_=fnb.rearrange("c p -> p c"))

            cls = wk.tile([128, KC, NIMG], F32, tag="cls")
            for c in range(KC):
                nc.vector.tensor_copy(out=cls[:, c, :],
                                      in_=tok[c][:, 0:T:NTOK])

            def head_ln(src, dst_fn, apply_affine):
                """LN over features for [768, NIMG] (src[kc] -> [128, NIMG])."""
                inv = 1.0 / D
                xb, xs = [], []
                for kc in range(KC):
                    b = wk.tile([128, NIMG], BF16, tag="hlnb", bufs=8)
                    nc.gpsimd.tensor_copy(out=b, in_=src(kc))
                    s = wk.tile([128, NIMG], BF16, tag="hlns", bufs=8)
                    nc.scalar.activation(out=s, in_=src(kc), func=AF.Square)
                    xb.append(b)
                    xs.append(s)
                psum = ps.tile([1, NIMG], F32, tag="attps", bufs=4)
                psq = ps.tile([1, NIMG], F32, tag="attps", bufs=4)
                for kc in range(KC):
                    nc.tensor.matmul(psum, ones_bf, xb[kc], start=(kc == 0),
                                     stop=(kc == KC - 1))
                for kc in range(KC):
                    nc.tensor.matmul(psq, ones_bf, xs[kc], start=(kc == 0),
                                     stop=(kc == KC - 1))
                st = wk.tile([1, 2, NIMG], F32, tag="hlnst", bufs=2)
                nc.scalar.mul(out=st[:, 0, :], in_=psum[:, :], mul=inv)
                m2 = wk.tile([1, NIMG], F32, tag="hlnm2", bufs=2)
                nc.vector.tensor_mul(m2, st[:, 0, :], st[:, 0, :])
                var = wk.tile([1, NIMG], F32, tag="hlnvar", bufs=2)
                nc.vector.scalar_tensor_tensor(out=var, in0=psq[:, :],
                                               scalar=inv, in1=m2,
                                               op0=OP.mult, op1=OP.subtract)
                nc.scalar.activation(out=st[:, 1, :], in_=var, func=AF.Sqrt,
                                     bias=eps_t)
                nc.vector.reciprocal(st[:, 1, :], st[:, 1, :])
                bc = wk.tile([128, 2, NIMG], F32, tag="hlnbc", bufs=2)
                nc.gpsimd.partition_broadcast(bc, st[0:1])
                for kc in range(KC):
                    tmp = wk.tile([128, NIMG], F32, tag="hlntmp", bufs=8)
                    nc.vector.tensor_sub(tmp, src(kc), bc[:, 0, :])
                    dst_fn(kc, tmp, bc)

            cls2 = wk.tile([128, KC, NIMG], F32, tag="cls2")

            def fn_dst(kc, tmp, bc):
                nc.vector.tensor_mul(tmp, tmp, bc[:, 1, :])
                nc.vector.tensor_scalar(
                    out=cls2[:, kc, :], in0=tmp, scalar1=fnw_t[:, kc:kc + 1],
                    scalar2=fnb_t[:, kc:kc + 1], op0=OP.mult, op1=OP.add)

            head_ln(lambda kc: cls[:, kc, :], fn_dst, True)

            clsr = wk.tile([128, KC, NIMG], F32R, tag="clsr")

            def hln_dst(kc, tmp, bc):
                nc.vector.tensor_mul(clsr[:, kc, :], tmp, bc[:, 1, :])

            head_ln(lambda kc: cls2[:, kc, :], hln_dst, False)

            # h1 GEMM + GELU (feature-major out [3072, NIMG])
            h1b_t = wk.tile([128, MC], F32, tag="h1b")
            nc.sync.dma_start(out=h1b_t, in_=h1b.rearrange("c p -> p c"))
            gh = wk.tile([128, MC, NIMG], F32R, tag="gh")
            for g0 in range(0, MC, 4):
                gsz = min(4, MC - g0)
                slabs = []
                for kc in range(KC):
                    wt = wp.tile([128, gsz * 128], F32R, tag="wslab", bufs=8)
                    nc.sync.dma_start(
                        out=wt,
                        in_=h1w[kc][:, g0 * 128:(g0 + gsz) * 128].bitcast(F32R))
                    slabs.append(wt)
                for dc in range(gsz):
                    p = ps.tile([128, NIMG], F32, tag="gemm", bufs=4)
                    for kc in range(KC):
                        nc.tensor.matmul(p, slabs[kc][:, dc * 128:(dc + 1) * 128],
                                         clsr[:, kc, :],
                                         start=(kc == 0), stop=(kc == KC - 1))
                    nc.scalar.activation(out=gh[:, g0 + dc, :], in_=p,
                                         func=AF.Gelu,
                                         bias=h1b_t[:, g0 + dc:g0 + dc + 1])

            # h2 GEMM -> logits [2, NIMG]
            h2w_t = wk.tile([128, MC, 2], F32R, tag="h2w")
            nc.sync.dma_start(out=h2w_t,
                              in_=h2w.rearrange("c p n -> p c n").bitcast(F32R))
            h2b_t = wk.tile([2, 1], F32, tag="h2b")
            nc.sync.dma_start(out=h2b_t, in_=h2b[:, :])
            pl = ps.tile([2, NIMG], F32, tag="attps", bufs=4)
            for mc in range(MC):
                nc.tensor.matmul(pl, h2w_t[:, mc, :], gh[:, mc, :],
                                 start=(mc == 0), stop=(mc == MC - 1))
            logits = wk.tile([2, NIMG], F32, tag="logits")
            nc.vector.tensor_scalar_add(out=logits, in0=pl, scalar1=h2b_t)
            lap = logits[:, :]
            oap = out[:, :]
            dst = bass.AP(tensor=oap.tensor, offset=oap.offset,
                          ap=[[1, 2], [2, NIMG]])
            nc.sync.dma_start(out=dst, in_=lap)

    nc.finalize()
    return nc


def _prep_host(inputs):
    """Host-side weight/activation layout prep. Returns (shared, per_core)."""
    f = np.float32
    x = np.asarray(inputs["x"], f)
    B = x.shape[0]
    # patch matrix: d=(c,kh,kw) major -> [768, B, 196]
    xp = x.reshape(B, 3, 14, 16, 14, 16)
    pats = np.ascontiguousarray(xp.transpose(1, 3, 5, 0, 2, 4)).reshape(768, B, 196)

    conv_w = np.asarray(inputs["conv_w"], f)          # [D, 3, 16, 16]
    convw = np.ascontiguousarray(conv_w.reshape(D, 768).T).reshape(KC, 128, D)
    convb = np.asarray(inputs["conv_b"], f).reshape(KC, 128)

    cls_t = np.asarray(inputs["cls_token"], f).reshape(D)
    pos_e = np.asarray(inputs["pos_embed"], f).reshape(NTOK, D)
    clspos = (cls_t + pos_e[0]).reshape(KC, 128)
    pos = np.ascontiguousarray(pos_e.T).reshape(KC, 128, NTOK)

    ln1w = np.asarray(inputs["ln1_w"], f)
    ln1b = np.asarray(inputs["ln1_b"], f)
    ln2w = np.asarray(inputs["ln2_w"], f)
    ln2b = np.asarray(inputs["ln2_b"], f)
    wq, wk_, wv = (np.asarray(inputs[k], f) for k in ("wq", "wk", "wv"))
    wqkv = np.concatenate([wq, wk_, wv], axis=2)      # [L, D, 3D]
    qkvw = ln1w[:, :, None] * wqkv
    qkvb = np.einsum("ld,ldo->lo", ln1b, wqkv)
    qkvw = np.ascontiguousarray(qkvw).reshape(DEPTH, KC, 128, 3 * D)
    qkvb = np.ascontiguousarray(qkvb).reshape(DEPTH, 3 * KC, 128)

    wow = np.ascontiguousarray(np.asarray(inputs["wo_w"], f)).reshape(
        DEPTH, KC, 128, D)
    wob = np.asarray(inputs["wo_b"], f).reshape(DEPTH, KC, 128)

    m1 = np.asarray(inputs["mlp1_w"], f)              # [L, D, MLP]
    m1w = np.ascontiguousarray(ln2w[:, :, None] * m1).reshape(DEPTH, KC, 128, MLP)
    m1b = (np.asarray(inputs["mlp1_b"], f) +
           np.einsum("ld,ldo->lo", ln2b, m1)).reshape(DEPTH, MC, 128)
    m2w = np.ascontiguousarray(np.asarray(inputs["mlp2_w"], f)).reshape(
        DEPTH, MC, 128, D)
    m2b = np.asarray(inputs["mlp2_b"], f).reshape(DEPTH, KC, 128)

    fnw = np.asarray(inputs["fnorm_w"], f).reshape(KC, 128)
    fnb = np.asarray(inputs["fnorm_b"], f).reshape(KC, 128)
    hlnw = np.asarray(inputs["hln_w"], f)
    hlnb = np.asarray(inputs["hln_b"], f)
    h1 = np.asarray(inputs["h1_w"], f)                # [D, MLP]
    h1w = np.ascontiguousarray(hlnw[:, None] * h1).reshape(KC, 128, MLP)
    h1b = (np.asarray(inputs["h1_b"], f) + hlnb @ h1).reshape(MC, 128)
    h2w = np.ascontiguousarray(np.asarray(inputs["h2_w"], f)).reshape(MC, 128, 2)
    h2b = np.asarray(inputs["h2_b"], f).reshape(2, 1)

    shared = dict(clspos=clspos, pos=pos, convw=convw, convb=convb,
                  qkvw=qkvw, qkvb=qkvb, wow=wow, wob=wob, m1w=m1w, m1b=m1b,
                  m2w=m2w, m2b=m2b, fnw=fnw, fnb=fnb, h1w=h1w, h1b=h1b,
                  h2w=h2w, h2b=h2b)
    per_core = []
    for c in range(NCORES):
        pc = np.ascontiguousarray(
            pats[:, c * NIMG:(c + 1) * NIMG, :]).reshape(KC, 128, NIMG * 196)
        per_core.append(dict(patches=pc, **shared))
    return per_core


def kernel(**inputs) -> np.ndarray:
    if "nc" not in _NC_CACHE:
        _NC_CACHE["nc"] = _build_nc()
    nc = _NC_CACHE["nc"]
    in_maps = _prep_host(inputs)
    trace = bool(int(os.environ.get("VIT_TRACE", "0")))
    res = run_bass_kernel_spmd(nc, in_maps, core_ids=list(range(NCORES)),
                               trace=trace)
    if trace:
        _NC_CACHE["last_exec_time_ns"] = res.exec_time_ns
        _NC_CACHE["last_result"] = res
    out = np.concatenate([res.results[c]["out"] for c in range(NCORES)], axis=0)
    return out.astype(np.float32)


# revision 9
# speedup vs baseline: 1.1481x; 1.1481x over previous
"""ViT-Base/16 forward pass on 8 TRN2 NeuronCores (data-parallel over batch).

Layout: feature-major activations [768, T=788] per core (4 images x 197 tokens),
stored as 6 chunks of [128, .] split into two token halves of 394 (f32r matmul
needs an even moving dim). GEMMs run in float32r (full PE rate); attention and
mlp2 internals run in bf16. LayerNorm affine params are folded into the
following GEMM weights host-side. Attention computes scores transposed
(K^T Q) so softmax'd probabilities feed the V-contraction directly without
transposing the attention matrix.
"""
import os
import numpy as np

import concourse.bass as bass
import concourse.bacc as bacc
import concourse.tile as tile
from concourse import mybir
from concourse.bass_utils import run_bass_kernel_spmd
from concourse.masks import make_identity

F32 = mybir.dt.float32
F32R = mybir.dt.float32r
BF16 = mybir.dt.bfloat16
AF = mybir.ActivationFunctionType
OP = mybir.AluOpType

D, DEPTH, H, DK, MLP = 768, 12, 12, 64, 3072
NIMG = 4            # images per core
NTOK = 197          # tokens per image
T = NIMG * NTOK     # 788
HALF = T // 2       # 394
KC = D // 128       # 6
MC = MLP // 128     # 24
SCALE = DK ** -0.5  # 0.125
EPS = 1e-5
NCORES = 8

_NC_CACHE = {}


def _build_nc():
    nc = bacc.Bacc()
    dp = nc.declare_dram_parameter

    patches = dp("patches", [KC, 128, NIMG * 196], F32, isOutput=False)
    clspos = dp("clspos", [KC, 128], F32, isOutput=False)
    pos = dp("pos", [KC, 128, NTOK], F32, isOutput=False)
    convw = dp("convw", [KC, 128, D], F32, isOutput=False)
    convb = dp("convb", [KC, 128], F32, isOutput=False)
    qkvw = dp("qkvw", [DEPTH, KC, 128, 3 * D], F32, isOutput=False)
    qkvb = dp("qkvb", [DEPTH, 3 * KC, 128], F32, isOutput=False)
    wow = dp("wow", [DEPTH, KC, 128, D], F32, isOutput=False)
    wob = dp("wob", [DEPTH, KC, 128], F32, isOutput=False)
    m1w = dp("m1w", [DEPTH, KC, 128, MLP], F32, isOutput=False)
    m1b = dp("m1b", [DEPTH, MC, 128], F32, isOutput=False)
    m2w = dp("m2w", [DEPTH, MC, 128, D], BF16, isOutput=False)
    m2b = dp("m2b", [DEPTH, KC, 128], F32, isOutput=False)
    fnw = dp("fnw", [KC, 128], F32, isOutput=False)
    fnb = dp("fnb", [KC, 128], F32, isOutput=False)
    h1w = dp("h1w", [KC, 128, MLP], F32, isOutput=False)
    h1b = dp("h1b", [MC, 128], F32, isOutput=False)
    h2w = dp("h2w", [MC, 128, 2], F32, isOutput=False)
    h2b = dp("h2b", [2, 1], F32, isOutput=False)
    out = dp("out", [NIMG, 2], F32, isOutput=True)

    with tile.TileContext(nc) as tc:
        with (
            tc.tile_pool(name="persist", bufs=1) as pp,
            tc.tile_pool(name="work", bufs=1) as wk,
            tc.tile_pool(name="wpool", bufs=1) as wp,
            tc.tile_pool(name="ps", bufs=1, space="PSUM") as ps,
        ):
            # ---- constants ----
            ident = pp.tile([128, 128], BF16)
            make_identity(nc, ident)
            ones_bf = pp.tile([128, 1], BF16)
            nc.vector.memset(ones_bf, 1.0)
            eps_t = pp.tile([1, 1], F32)
            nc.vector.memset(eps_t, EPS)

            pos_t = [pp.tile([128, NTOK], F32, tag=f"pos{c}", name=f"pos{c}")
                     for c in range(KC)]
            for c in range(KC):
                nc.sync.dma_start(out=pos_t[c], in_=pos[c, :, :])

            # persistent activations, per (chunk, half): [128, HALF]
            def halfpair(base, dt):
                return [[pp.tile([128, HALF], dt, tag=f"{base}{c}h{h}",
                                 name=f"{base}{c}h{h}") for h in range(2)]
                        for c in range(KC)]

            tok = halfpair("tok", F32)
            q_bf = halfpair("q", BF16)
            k_bf = halfpair("k", BF16)
            v_bf = halfpair("v", BF16)
            ctx = halfpair("ctx", F32R)
            hmlp = pp.tile([128, MC, T], BF16, tag="hmlp")

            def gemm_smallk(w_src, n_dout, rhs_fn, consumer, nhalf=HALF,
                            G=4, kcn=KC, wdt=F32R):
                """w_src(kc)->dram AP [128, n_dout*128]; rhs_fn(kc,half)->sbuf
                [128,nhalf] AP; consumer(dc, half, psum)."""
                for g0 in range(0, n_dout, G):
                    gsz = min(G, n_dout - g0)
                    slabs = []
                    for kc in range(kcn):
                        wt = wp.tile([128, gsz * 128], wdt, tag="wslab", bufs=8)
                        src = w_src(kc)[:, g0 * 128:(g0 + gsz) * 128]
                        if wdt == F32R:
                            src = src.bitcast(F32R)
                        nc.sync.dma_start(out=wt, in_=src)
                        slabs.append(wt)
                    for half in range(2):
                        for dc in range(gsz):
                            p = ps.tile([128, nhalf], F32, tag="gemm", bufs=4)
                            for kc in range(kcn):
                                nc.tensor.matmul(
                                    p, slabs[kc][:, dc * 128:(dc + 1) * 128],
                                    rhs_fn(kc, half),
                                    start=(kc == 0), stop=(kc == kcn - 1))
                            consumer(g0 + dc, half, p)

            def layernorm(src_fn, dst):
                """Feature-major LN. src_fn(kc, half)->[128, HALF] fp32 AP;
                dst(kc, half, tmp, bc) consumes (x-mean) in tmp, rstd in
                bc[:, 1, :]."""
                inv = 1.0 / D
                for half in range(2):
                    xb, xs = [], []
                    for kc in range(KC):
                        b = wk.tile([128, HALF], BF16, tag="lnxb", bufs=6)
                        nc.gpsimd.tensor_copy(out=b, in_=src_fn(kc, half))
                        s = wk.tile([128, HALF], BF16, tag="lnxs", bufs=6)
                        nc.scalar.activation(out=s, in_=src_fn(kc, half),
                                             func=AF.Square)
                        xb.append(b)
                        xs.append(s)
                    psum = ps.tile([1, HALF], F32, tag="attps", bufs=4)
                    psq = ps.tile([1, HALF], F32, tag="attps", bufs=4)
                    for kc in range(KC):
                        nc.tensor.matmul(psum, ones_bf, xb[kc],
                                         start=(kc == 0), stop=(kc == KC - 1))
                    for kc in range(KC):
                        nc.tensor.matmul(psq, ones_bf, xs[kc],
                                         start=(kc == 0), stop=(kc == KC - 1))
                    st = wk.tile([1, 2, HALF], F32, tag="lnst", bufs=2)
                    nc.scalar.mul(out=st[:, 0, :], in_=psum[:, :], mul=inv)
                    m2 = wk.tile([1, HALF], F32, tag="lnm2", bufs=2)
                    nc.vector.tensor_mul(m2, st[:, 0, :], st[:, 0, :])
                    var = wk.tile([1, HALF], F32, tag="lnvar", bufs=2)
                    nc.vector.scalar_tensor_tensor(
                        out=var, in0=psq[:, :], scalar=inv, in1=m2,
                        op0=OP.mult, op1=OP.subtract)
                    nc.scalar.activation(out=st[:, 1, :], in_=var,
                                         func=AF.Sqrt, bias=eps_t)
                    nc.vector.reciprocal(st[:, 1, :], st[:, 1, :])
                    bc = wk.tile([128, 2, HALF], F32, tag="lnbc", bufs=2)
                    nc.gpsimd.partition_broadcast(bc, st[0:1])
                    for kc in range(KC):
                        tmp = wk.tile([128, HALF], F32, tag="lntmp", bufs=3)
                        nc.vector.tensor_sub(tmp, src_fn(kc, half),
                                             bc[:, 0, :])
                        dst(kc, half, tmp, bc)

            # ================= patch embed =================
            PW = 392  # half width of 784 patch cols
            with tc.tile_pool(name="patp", bufs=1) as patp:
                pat = [patp.tile([128, NIMG * 196], F32R, tag=f"pat{c}",
                                 name=f"pat{c}") for c in range(KC)]
                for c in range(KC):
                    nc.sync.dma_start(out=pat[c], in_=patches[c].bitcast(F32R))
                convb_t = wk.tile([128, KC], F32, tag="convb")
                nc.sync.dma_start(out=convb_t, in_=convb.rearrange("c p -> p c"))

                def conv_consumer(dc, half, p):
                    # psum cols [0:196]/[196:392] -> imgs 2*half / 2*half+1,
                    # local token cols j*197 + 1 .. of the half tile
                    for j in range(2):
                        nc.scalar.activation(
                            out=tok[dc][half][:, j * NTOK + 1:(j + 1) * NTOK],
                            in_=p[:, j * 196:(j + 1) * 196],
                            func=AF.Gelu, bias=convb_t[:, dc:dc + 1])

                gemm_smallk(lambda kc: convw[kc], KC,
                            lambda kc, half: pat[kc][:, half * PW:(half + 1) * PW],
                            conv_consumer, nhalf=PW)

            # cls columns + positional embedding
            for c in range(KC):
                for h in range(2):
                    tokap = tok[c][h][:, :]
                    dst = bass.AP(tensor=tokap.tensor, offset=tokap.offset,
                                  ap=[tokap.ap[0], [NTOK, 2]])
                    cpap = clspos[c, :]
                    src = bass.AP(tensor=cpap.tensor, offset=cpap.offset,
                                  ap=[cpap.ap[0], [0, 2]])
                    nc.sync.dma_start(out=dst, in_=src)
            for c in range(KC):
                for h in range(2):
                    for j in range(2):
                        nc.vector.tensor_add(
                            tok[c][h][:, j * NTOK + 1:(j + 1) * NTOK],
                            tok[c][h][:, j * NTOK + 1:(j + 1) * NTOK],
                            pos_t[c][:, 1:NTOK])

            # ================= transformer layers =================
            for layer in range(DEPTH):
                # ---- LN1 -> xhat (f32r) ----
                xhat = wk.tile([128, KC, 2, HALF], F32R, tag="xhat", bufs=1)

                def ln1_dst(kc, half, tmp, bc):
                    nc.vector.tensor_mul(xhat[:, kc, half, :], tmp, bc[:, 1, :])

                layernorm(lambda kc, half: tok[kc][half][:, :], ln1_dst)

                # ---- QKV ----
                qkvb_t = wk.tile([128, 3 * KC], F32, tag="qkvb", bufs=2)
                nc.sync.dma_start(out=qkvb_t,
                                  in_=qkvb[layer].rearrange("c p -> p c"))

                def qkv_consumer(dc, half, p):
                    dst = (q_bf, k_bf, v_bf)[dc // KC][dc % KC][half]
                    nc.vector.tensor_scalar_add(
                        out=dst, in0=p, scalar1=qkvb_t[:, dc:dc + 1])

                gemm_smallk(lambda kc: qkvw[layer, kc], 3 * KC,
                            lambda kc, half: xhat[:, kc, half, :],
                            qkv_consumer)

                # ---- attention (per image, per head; bf16, scoresT form) ----
                NSPLIT = ((0, 128), (128, NTOK))
                for img in range(NIMG):
                    ihalf, c0 = img // 2, (img % 2) * NTOK
                    for h in range(H):
                        ch, r0 = h // 2, (h % 2) * DK
                        qs = q_bf[ch][ihalf][r0:r0 + DK, c0:c0 + NTOK]
                        ks = k_bf[ch][ihalf][r0:r0 + DK, c0:c0 + NTOK]
                        vs = v_bf[ch][ihalf][r0:r0 + DK, c0:c0 + NTOK]
                        # scoresT chunks [m_sz, 197]; exp -> esT; V^T -> vT
                        esT, vT = [], []
                        pssum = ps.tile([1, NTOK], F32, tag="attps", bufs=4)
                        for mi, (m0, m1) in enumerate(NSPLIT):
                            msz = m1 - m0
                            sT = ps.tile([128, NTOK], F32, tag="attps", bufs=4)
                            nc.tensor.matmul(sT[0:msz], ks[:, m0:m1], qs,
                                             start=True, stop=True)
                            e = wk.tile([128, NTOK], BF16, tag=f"esT{mi}",
                                        bufs=2, name=f"esT{mi}")
                            nc.scalar.activation(out=e[0:msz], in_=sT[0:msz],
                                                 func=AF.Exp, scale=SCALE)
                            esT.append(e)
                            pv = ps.tile([128, DK], BF16, tag="attps", bufs=4)
                            nc.tensor.transpose(
                                pv[0:msz], vs[:, m0:m1],
                                ident[r0:r0 + DK, r0:r0 + DK])
                            vt = wk.tile([128, DK], BF16, tag=f"vT{mi}",
                                         bufs=2, name=f"vT{mi}")
                            nc.vector.tensor_copy(out=vt[0:msz], in_=pv[0:msz])
                            vT.append(vt)
                        for mi, (m0, m1) in enumerate(NSPLIT):
                            msz = m1 - m0
                            nc.tensor.matmul(pssum, ones_bf[0:msz],
                                             esT[mi][0:msz],
                                             start=(mi == 0), stop=(mi == 1))
                        rs = wk.tile([1, NTOK], F32, tag="rs", bufs=4)
                        nc.vector.reciprocal(rs, pssum[:, :])
                        rb = wk.tile([64, NTOK], F32, tag="rb", bufs=4)
                        nc.gpsimd.partition_broadcast(rb, rs)
                        pc = ps.tile([64, NTOK], F32, tag="attps", bufs=4)
                        for mi, (m0, m1) in enumerate(NSPLIT):
                            msz = m1 - m0
                            nc.tensor.matmul(pc, vT[mi][0:msz],
                                             esT[mi][0:msz],
                                             start=(mi == 0), stop=(mi == 1))
                        nc.vector.tensor_mul(
                            ctx[ch][ihalf][r0:r0 + DK, c0:c0 + NTOK], pc, rb)

                # ---- O projection + residual ----
                wob_t = wk.tile([128, KC], F32, tag="wob", bufs=2)
                nc.sync.dma_start(out=wob_t,
                                  in_=wob[layer].rearrange("c p -> p c"))

                def wo_consumer(dc, half, p):
                    sl = tok[dc][half][:, :]
                    nc.vector.scalar_tensor_tensor(
                        out=sl, in0=p, scalar=wob_t[:, dc:dc + 1], in1=sl,
                        op0=OP.add, op1=OP.add)

                gemm_smallk(lambda kc: wow[layer, kc], KC,
                            lambda kc, half: ctx[kc][half][:, :],
                            wo_consumer)

                # ---- LN2 -> xhat2 ----
                xhat2 = wk.tile([128, KC, 2, HALF], F32R, tag="xhat", bufs=1)

                def ln2_dst(kc, half, tmp, bc):
                    nc.vector.tensor_mul(xhat2[:, kc, half, :], tmp, bc[:, 1, :])

                layernorm(lambda kc, half: tok[kc][half][:, :], ln2_dst)

                # ---- MLP1 + GELU -> hmlp (bf16, both halves) ----
                m1b_t = wk.tile([128, MC], F32, tag="m1b", bufs=2)
                nc.sync.dma_start(out=m1b_t,
                                  in_=m1b[layer].rearrange("c p -> p c"))

                def m1_consumer(dc, half, p):
                    nc.scalar.activation(
                        out=hmlp[:, dc, half * HALF:(half + 1) * HALF],
                        in_=p, func=AF.Gelu, bias=m1b_t[:, dc:dc + 1])

                gemm_smallk(lambda kc: m1w[layer, kc], MC,
                            lambda kc, half: xhat2[:, kc, half, :],
                            m1_consumer)

                # ---- MLP2 (bf16 weights) + residual ----
                m2b_t = wk.tile([128, KC], F32, tag="m2b", bufs=2)
                nc.sync.dma_start(out=m2b_t,
                                  in_=m2b[layer].rearrange("c p -> p c"))

                def m2_consumer(dc, half, p):
                    sl = tok[dc][half][:, :]
                    nc.vector.scalar_tensor_tensor(
                        out=sl, in0=p, scalar=m2b_t[:, dc:dc + 1], in1=sl,
                        op0=OP.add, op1=OP.add)

                gemm_smallk(lambda kc: m2w[layer, kc], KC,
                            lambda kc, half:
                            hmlp[:, kc, half * HALF:(half + 1) * HALF],
                            m2_consumer, G=2, kcn=MC, wdt=BF16)

            # ================= final head (cls tokens only) =================
            fnw_t = wk.tile([128, KC], F32, tag="fnw")
            fnb_t = wk.tile([128, KC], F32, tag="fnb")
            nc.sync.dma_start(out=fnw_t, in_=fnw.rearrange("c p -> p c"))
            nc.sync.dma_start(out=fnb_t, in_=fnb.rearrange("c p -> p c"))

            cls = wk.tile([128, KC, NIMG], F32, tag="cls")
            for c in range(KC):
                for h in range(2):
                    nc.vector.tensor_copy(out=cls[:, c, 2 * h:2 * h + 2],
                                          in_=tok[c][h][:, 0:2 * NTOK:NTOK])

            def head_ln(src, dst_fn):
                inv = 1.0 / D
                xb, xs = [], []
                for kc in range(KC):
                    b = wk.tile([128, NIMG], BF16, tag="hlnb", bufs=8)
                    nc.gpsimd.tensor_copy(out=b, in_=src(kc))
                    s = wk.tile([128, NIMG], BF16, tag="hlns", bufs=8)
                    nc.scalar.activation(out=s, in_=src(kc), func=AF.Square)
                    xb.append(b)
                    xs.append(s)
                psum = ps.tile([1, NIMG], F32, tag="attps", bufs=4)
                psq = ps.tile([1, NIMG], F32, tag="attps", bufs=4)
                for kc in range(KC):
                    nc.tensor.matmul(psum, ones_bf, xb[kc], start=(kc == 0),
                                     stop=(kc == KC - 1))
                for kc in range(KC):
                    nc.tensor.matmul(psq, ones_bf, xs[kc], start=(kc == 0),
                                     stop=(kc == KC - 1))
                st = wk.tile([1, 2, NIMG], F32, tag="hlnst", bufs=2)
                nc.scalar.mul(out=st[:, 0, :], in_=psum[:, :], mul=inv)
                m2 = wk.tile([1, NIMG], F32, tag="hlnm2", bufs=2)
                nc.vector.tensor_mul(m2, st[:, 0, :], st[:, 0, :])
                var = wk.tile([1, NIMG], F32, tag="hlnvar", bufs=2)
                nc.vector.scalar_tensor_tensor(out=var, in0=psq[:, :],
                                               scalar=inv, in1=m2,
                                               op0=OP.mult, op1=OP.subtract)
                nc.scalar.activation(out=st[:, 1, :], in_=var, func=AF.Sqrt,
                                     bias=eps_t)
                nc.vector.reciprocal(st[:, 1, :], st[:, 1, :])
                bc = wk.tile([128, 2, NIMG], F32, tag="hlnbc", bufs=2)
                nc.gpsimd.partition_broadcast(bc, st[0:1])
                for kc in range(KC):
                    tmp = wk.tile([128, NIMG], F32, tag="hlntmp", bufs=8)
                    nc.vector.tensor_sub(tmp, src(kc), bc[:, 0, :])
                    dst_fn(kc, tmp, bc)

            cls2 = wk.tile([128, KC, NIMG], F32, tag="cls2")

            def fn_dst(kc, tmp, bc):
                nc.vector.tensor_mul(tmp, tmp, bc[:, 1, :])
                nc.vector.tensor_scalar(
                    out=cls2[:, kc, :], in0=tmp, scalar1=fnw_t[:, kc:kc + 1],
                    scalar2=fnb_t[:, kc:kc + 1], op0=OP.mult, op1=OP.add)

            head_ln(lambda kc: cls[:, kc, :], fn_dst)

            clsr = wk.tile([128, KC, NIMG], F32R, tag="clsr")

            def hln_dst(kc, tmp, bc):
                nc.vector.tensor_mul(clsr[:, kc, :], tmp, bc[:, 1, :])

            head_ln(lambda kc: cls2[:, kc, :], hln_dst)

            # h1 GEMM + GELU (feature-major out [3072, NIMG])
            h1b_t = wk.tile([128, MC], F32, tag="h1b")
            nc.sync.dma_start(out=h1b_t, in_=h1b.rearrange("c p -> p c"))
            gh = wk.tile([128, MC, NIMG], F32R, tag="gh")
            for g0 in range(0, MC, 4):
                gsz = min(4, MC - g0)
                slabs = []
                for kc in range(KC):
                    wt = wp.tile([128, gsz * 128], F32R, tag="wslab", bufs=8)
                    nc.sync.dma_start(
                        out=wt,
                        in_=h1w[kc][:, g0 * 128:(g0 + gsz) * 128].bitcast(F32R))
                    slabs.append(wt)
                for dc in range(gsz):
                    p = ps.tile([128, NIMG], F32, tag="gemm", bufs=4)
                    for kc in range(KC):
                        nc.tensor.matmul(p, slabs[kc][:, dc * 128:(dc + 1) * 128],
                                         clsr[:, kc, :],
                                         start=(kc == 0), stop=(kc == KC - 1))
                    nc.scalar.activation(out=gh[:, g0 + dc, :], in_=p,
                                         func=AF.Gelu,
                                         bias=h1b_t[:, g0 + dc:g0 + dc + 1])

            # h2 GEMM -> logits [2, NIMG]
            h2w_t = wk.tile([128, MC, 2], F32R, tag="h2w")
            nc.sync.dma_start(out=h2w_t,
                              in_=h2w.rearrange("c p n -> p c n").bitcast(F32R))
            h2b_t = wk.tile([2, 1], F32, tag="h2b")
            nc.sync.dma_start(out=h2b_t, in_=h2b[:, :])
            pl = ps.tile([2, NIMG], F32, tag="attps", bufs=4)
            for mc in range(MC):
                nc.tensor.matmul(pl, h2w_t[:, mc, :], gh[:, mc, :],
                                 start=(mc == 0), stop=(mc == MC - 1))
            logits = wk.tile([2, NIMG], F32, tag="logits")
            nc.vector.tensor_scalar_add(out=logits, in0=pl, scalar1=h2b_t)
            lap = logits[:, :]
            oap = out[:, :]
            dst = bass.AP(tensor=oap.tensor, offset=oap.offset,
                          ap=[[1, 2], [2, NIMG]])
            nc.sync.dma_start(out=dst, in_=lap)

    nc.finalize()
    return nc


def _prep_host(inputs):
    """Host-side weight/activation layout prep. Returns per-core input maps."""
    import ml_dtypes
    f = np.float32
    x = np.asarray(inputs["x"], f)
    B = x.shape[0]
    # patch matrix: d=(c,kh,kw) major -> [768, B, 196]
    xp = x.reshape(B, 3, 14, 16, 14, 16)
    pats = np.ascontiguousarray(xp.transpose(1, 3, 5, 0, 2, 4)).reshape(768, B, 196)

    conv_w = np.asarray(inputs["conv_w"], f)          # [D, 3, 16, 16]
    convw = np.ascontiguousarray(conv_w.reshape(D, 768).T).reshape(KC, 128, D)
    convb = np.asarray(inputs["conv_b"], f).reshape(KC, 128)

    cls_t = np.asarray(inputs["cls_token"], f).reshape(D)
    pos_e = np.asarray(inputs["pos_embed"], f).reshape(NTOK, D)
    clspos = (cls_t + pos_e[0]).reshape(KC, 128)
    pos = np.ascontiguousarray(pos_e.T).reshape(KC, 128, NTOK)

    ln1w = np.asarray(inputs["ln1_w"], f)
    ln1b = np.asarray(inputs["ln1_b"], f)
    ln2w = np.asarray(inputs["ln2_w"], f)
    ln2b = np.asarray(inputs["ln2_b"], f)
    wq, wk_, wv = (np.asarray(inputs[k], f) for k in ("wq", "wk", "wv"))
    wqkv = np.concatenate([wq, wk_, wv], axis=2)      # [L, D, 3D]
    qkvw = ln1w[:, :, None] * wqkv
    qkvb = np.einsum("ld,ldo->lo", ln1b, wqkv)
    qkvw = np.ascontiguousarray(qkvw).reshape(DEPTH, KC, 128, 3 * D)
    qkvb = np.ascontiguousarray(qkvb).reshape(DEPTH, 3 * KC, 128)

    wow = np.ascontiguousarray(np.asarray(inputs["wo_w"], f)).reshape(
        DEPTH, KC, 128, D)
    wob = np.asarray(inputs["wo_b"], f).reshape(DEPTH, KC, 128)

    m1 = np.asarray(inputs["mlp1_w"], f)              # [L, D, MLP]
    m1w = np.ascontiguousarray(ln2w[:, :, None] * m1).reshape(DEPTH, KC, 128, MLP)
    m1b = (np.asarray(inputs["mlp1_b"], f) +
           np.einsum("ld,ldo->lo", ln2b, m1)).reshape(DEPTH, MC, 128)
    m2w = np.ascontiguousarray(np.asarray(inputs["mlp2_w"], f)).reshape(
        DEPTH, MC, 128, D).astype(ml_dtypes.bfloat16)
    m2b = np.asarray(inputs["mlp2_b"], f).reshape(DEPTH, KC, 128)

    fnw = np.asarray(inputs["fnorm_w"], f).reshape(KC, 128)
    fnb = np.asarray(inputs["fnorm_b"], f).reshape(KC, 128)
    hlnw = np.asarray(inputs["hln_w"], f)
    hlnb = np.asarray(inputs["hln_b"], f)
    h1 = np.asarray(inputs["h1_w"], f)                # [D, MLP]
    h1w = np.ascontiguousarray(hlnw[:, None] * h1).reshape(KC, 128, MLP)
    h1b = (np.asarray(inputs["h1_b"], f) + hlnb @ h1).reshape(MC, 128)
    h2w = np.ascontiguousarray(np.asarray(inputs["h2_w"], f)).reshape(MC, 128, 2)
    h2b = np.asarray(inputs["h2_b"], f).reshape(2, 1)

    shared = dict(clspos=clspos, pos=pos, convw=convw, convb=convb,
                  qkvw=qkvw, qkvb=qkvb, wow=wow, wob=wob, m1w=m1w, m1b=m1b,
                  m2w=m2w, m2b=m2b, fnw=fnw, fnb=fnb, h1w=h1w, h1b=h1b,
                  h2w=h2w, h2b=h2b)
    per_core = []
    for c in range(NCORES):
        pc = np.ascontiguousarray(
            pats[:, c * NIMG:(c + 1) * NIMG, :]).reshape(KC, 128, NIMG * 196)
        per_core.append(dict(patches=pc, **shared))
    return per_core


def kernel(**inputs) -> np.ndarray:
    if "nc" not in _NC_CACHE:
        _NC_CACHE["nc"] = _build_nc()
    nc = _NC_CACHE["nc"]
    in_maps = _prep_host(inputs)
    trace = bool(int(os.environ.get("VIT_TRACE", "0")))
    res = run_bass_kernel_spmd(nc, in_maps, core_ids=list(range(NCORES)),
                               trace=trace)
    if trace:
        _NC_CACHE["last_exec_time_ns"] = res.exec_time_ns
        _NC_CACHE["last_result"] = res
    out = np.concatenate([res.results[c]["out"] for c in range(NCORES)], axis=0)
    return out.astype(np.float32)


# revision 11
# speedup vs baseline: 1.2913x; 1.1247x over previous
"""ViT-Base/16 forward pass on 8 TRN2 NeuronCores (data-parallel over batch).

Layout: feature-major activations [768, T=788] per core (4 images x 197 tokens),
stored as 6 chunks of [128, .] split into two token halves of 394 (f32r matmul
needs an even moving dim). GEMMs run in float32r (full PE rate); attention and
mlp2 internals run in bf16. LayerNorm affine params are folded into the
following GEMM weights host-side. Attention computes scores transposed
(K^T Q) so softmax'd probabilities feed the V-contraction directly without
transposing the attention matrix.
"""
import os
import numpy as np

import concourse.bass as bass
import concourse.bacc as bacc
import concourse.tile as tile
from concourse import mybir
from concourse.bass_utils import run_bass_kernel_spmd
from concourse.masks import make_identity

F32 = mybir.dt.float32
F32R = mybir.dt.float32r
BF16 = mybir.dt.bfloat16
AF = mybir.ActivationFunctionType
OP = mybir.AluOpType

D, DEPTH, H, DK, MLP = 768, 12, 12, 64, 3072
NIMG = 4            # images per core
NTOK = 197          # tokens per image
T = NIMG * NTOK     # 788
HALF = T // 2       # 394
KC = D // 128       # 6
MC = MLP // 128     # 24
SCALE = DK ** -0.5  # 0.125
EPS = 1e-5
NCORES = 8

_NC_CACHE = {}


def _build_nc():
    nc = bacc.Bacc()
    dp = nc.declare_dram_parameter

    patches = dp("patches", [KC, 128, NIMG * 196], F32, isOutput=False)
    clspos = dp("clspos", [KC, 128], F32, isOutput=False)
    pos = dp("pos", [KC, 128, NTOK], F32, isOutput=False)
    convw = dp("convw", [KC, 128, D], F32, isOutput=False)
    convb = dp("convb", [KC, 128], F32, isOutput=False)
    qkvw = dp("qkvw", [DEPTH, KC, 128, 3 * D], F32, isOutput=False)
    qkvb = dp("qkvb", [DEPTH, 3 * KC, 128], F32, isOutput=False)
    wow = dp("wow", [DEPTH, KC, 128, D], F32, isOutput=False)
    wob = dp("wob", [DEPTH, KC, 128], F32, isOutput=False)
    m1w = dp("m1w", [DEPTH, KC, 128, MLP], F32, isOutput=False)
    m1b = dp("m1b", [DEPTH, MC, 128], F32, isOutput=False)
    m2w = dp("m2w", [DEPTH, MC, 128, D], BF16, isOutput=False)
    m2b = dp("m2b", [DEPTH, KC, 128], F32, isOutput=False)
    fnw = dp("fnw", [KC, 128], F32, isOutput=False)
    fnb = dp("fnb", [KC, 128], F32, isOutput=False)
    h1w = dp("h1w", [KC, 128, MLP], F32, isOutput=False)
    h1b = dp("h1b", [MC, 128], F32, isOutput=False)
    h2w = dp("h2w", [MC, 128, 2], F32, isOutput=False)
    h2b = dp("h2b", [2, 1], F32, isOutput=False)
    out = dp("out", [NIMG, 2], F32, isOutput=True)

    with tile.TileContext(nc) as tc:
        with (
            tc.tile_pool(name="persist", bufs=1) as pp,
            tc.tile_pool(name="work", bufs=1) as wk,
            tc.tile_pool(name="wpool", bufs=1) as wp,
            tc.tile_pool(name="ps", bufs=1, space="PSUM") as ps,
        ):
            # ---- constants ----
            ident = pp.tile([128, 128], BF16)
            make_identity(nc, ident)
            ones_bf = pp.tile([128, 1], BF16)
            nc.vector.memset(ones_bf, 1.0)
            eps_t = pp.tile([1, 1], F32)
            nc.vector.memset(eps_t, EPS)

            pos_t = [pp.tile([128, NTOK], F32, tag=f"pos{c}", name=f"pos{c}")
                     for c in range(KC)]
            for c in range(KC):
                nc.sync.dma_start(out=pos_t[c], in_=pos[c, :, :])

            # persistent activations, per (chunk, half): [128, HALF]
            def halfpair(base, dt):
                return [[pp.tile([128, HALF], dt, tag=f"{base}{c}h{h}",
                                 name=f"{base}{c}h{h}") for h in range(2)]
                        for c in range(KC)]

            tok = halfpair("tok", F32)
            q_bf = halfpair("q", BF16)
            k_bf = halfpair("k", BF16)
            v_bf = halfpair("v", BF16)
            ctx = halfpair("ctx", F32R)
            hmlp = pp.tile([128, MC, T], BF16, tag="hmlp")

            def gemm_smallk(w_src, n_dout, rhs_fn, consumer, nhalf=HALF,
                            G=4, kcn=KC, wdt=F32R):
                """w_src(kc)->dram AP [128, n_dout*128]; rhs_fn(kc,half)->sbuf
                [128,nhalf] AP; consumer(dc, half, psum)."""
                for g0 in range(0, n_dout, G):
                    gsz = min(G, n_dout - g0)
                    slabs = []
                    for kc in range(kcn):
                        wt = wp.tile([128, gsz * 128], wdt, tag="wslab", bufs=8)
                        src = w_src(kc)[:, g0 * 128:(g0 + gsz) * 128]
                        if wdt == F32R:
                            src = src.bitcast(F32R)
                        nc.sync.dma_start(out=wt, in_=src)
                        slabs.append(wt)
                    for half in range(2):
                        for dc in range(gsz):
                            p = ps.tile([128, nhalf], F32, tag="ps", bufs=8)
                            for kc in range(kcn):
                                nc.tensor.matmul(
                                    p, slabs[kc][:, dc * 128:(dc + 1) * 128],
                                    rhs_fn(kc, half),
                                    start=(kc == 0), stop=(kc == kcn - 1))
                            consumer(g0 + dc, half, p)

            def layernorm(src_fn, dst):
                """Feature-major LN. src_fn(kc, half)->[128, HALF] fp32 AP;
                dst(kc, half, tmp, bc) consumes (x-mean) in tmp, rstd in
                bc[:, 1, :]."""
                inv = 1.0 / D
                for half in range(2):
                    xb, xs = [], []
                    for kc in range(KC):
                        b = wk.tile([128, HALF], BF16, tag="lnxb", bufs=4)
                        nc.gpsimd.tensor_copy(out=b, in_=src_fn(kc, half))
                        s = wk.tile([128, HALF], BF16, tag="lnxs", bufs=4)
                        nc.scalar.activation(out=s, in_=src_fn(kc, half),
                                             func=AF.Square)
                        xb.append(b)
                        xs.append(s)
                    psum = ps.tile([1, HALF], F32, tag="ps", bufs=8)
                    psq = ps.tile([1, HALF], F32, tag="ps", bufs=8)
                    for kc in range(KC):
                        nc.tensor.matmul(psum, ones_bf, xb[kc],
                                         start=(kc == 0), stop=(kc == KC - 1))
                    for kc in range(KC):
                        nc.tensor.matmul(psq, ones_bf, xs[kc],
                                         start=(kc == 0), stop=(kc == KC - 1))
                    st = wk.tile([1, 2, HALF], F32, tag="lnst", bufs=2)
                    nc.scalar.mul(out=st[:, 0, :], in_=psum[:, :], mul=inv)
                    m2 = wk.tile([1, HALF], F32, tag="lnm2", bufs=2)
                    nc.vector.tensor_mul(m2, st[:, 0, :], st[:, 0, :])
                    var = wk.tile([1, HALF], F32, tag="lnvar", bufs=2)
                    nc.vector.scalar_tensor_tensor(
                        out=var, in0=psq[:, :], scalar=inv, in1=m2,
                        op0=OP.mult, op1=OP.subtract)
                    nc.scalar.activation(out=st[:, 1, :], in_=var,
                                         func=AF.Sqrt, bias=eps_t)
                    nc.vector.reciprocal(st[:, 1, :], st[:, 1, :])
                    bc = wk.tile([128, 2, HALF], F32, tag="lnbc", bufs=2)
                    nc.gpsimd.partition_broadcast(bc, st[0:1])
                    for kc in range(KC):
                        tmp = wk.tile([128, HALF], F32, tag="lntmp", bufs=2)
                        nc.vector.tensor_sub(tmp, src_fn(kc, half),
                                             bc[:, 0, :])
                        dst(kc, half, tmp, bc)

            # ================= patch embed =================
            PW = 392  # half width of 784 patch cols
            with tc.tile_pool(name="patp", bufs=1) as patp:
                pat = [patp.tile([128, NIMG * 196], F32R, tag=f"pat{c}",
                                 name=f"pat{c}") for c in range(KC)]
                for c in range(KC):
                    nc.sync.dma_start(out=pat[c], in_=patches[c].bitcast(F32R))
                convb_t = wk.tile([128, KC], F32, tag="convb")
                nc.sync.dma_start(out=convb_t, in_=convb.rearrange("c p -> p c"))

                def conv_consumer(dc, half, p):
                    # psum cols [0:196]/[196:392] -> imgs 2*half / 2*half+1,
                    # local token cols j*197 + 1 .. of the half tile
                    for j in range(2):
                        nc.scalar.activation(
                            out=tok[dc][half][:, j * NTOK + 1:(j + 1) * NTOK],
                            in_=p[:, j * 196:(j + 1) * 196],
                            func=AF.Gelu, bias=convb_t[:, dc:dc + 1])

                gemm_smallk(lambda kc: convw[kc], KC,
                            lambda kc, half: pat[kc][:, half * PW:(half + 1) * PW],
                            conv_consumer, nhalf=PW)

            # cls columns + positional embedding
            for c in range(KC):
                for h in range(2):
                    tokap = tok[c][h][:, :]
                    dst = bass.AP(tensor=tokap.tensor, offset=tokap.offset,
                                  ap=[tokap.ap[0], [NTOK, 2]])
                    cpap = clspos[c, :]
                    src = bass.AP(tensor=cpap.tensor, offset=cpap.offset,
                                  ap=[cpap.ap[0], [0, 2]])
                    nc.sync.dma_start(out=dst, in_=src)
            for c in range(KC):
                for h in range(2):
                    for j in range(2):
                        nc.vector.tensor_add(
                            tok[c][h][:, j * NTOK + 1:(j + 1) * NTOK],
                            tok[c][h][:, j * NTOK + 1:(j + 1) * NTOK],
                            pos_t[c][:, 1:NTOK])

            # ================= transformer layers =================
            for layer in range(DEPTH):
                # ---- LN1 -> xhat (f32r) ----
                xhat = wk.tile([128, KC, 2, HALF], F32R, tag="xhat", bufs=1)

                def ln1_dst(kc, half, tmp, bc):
                    nc.vector.tensor_mul(xhat[:, kc, half, :], tmp, bc[:, 1, :])

                layernorm(lambda kc, half: tok[kc][half][:, :], ln1_dst)

                # ---- QKV ----
                qkvb_t = wk.tile([128, 3 * KC], F32, tag="qkvb", bufs=2)
                nc.sync.dma_start(out=qkvb_t,
                                  in_=qkvb[layer].rearrange("c p -> p c"))

                def qkv_consumer(dc, half, p):
                    dst = (q_bf, k_bf, v_bf)[dc // KC][dc % KC][half]
                    nc.vector.tensor_scalar_add(
                        out=dst, in0=p, scalar1=qkvb_t[:, dc:dc + 1])

                gemm_smallk(lambda kc: qkvw[layer, kc], 3 * KC,
                            lambda kc, half: xhat[:, kc, half, :],
                            qkv_consumer)

                # ---- attention (per image, per head; bf16, scoresT form) ----
                NSPLIT = ((0, 128), (128, NTOK))
                for img in range(NIMG):
                    ihalf, c0 = img // 2, (img % 2) * NTOK
                    # batched V transpose: [128(2 heads), msz] -> [msz, 128]
                    vT2 = [[None] * KC for _ in range(2)]
                    for mi, (m0, m1) in enumerate(NSPLIT):
                        msz = m1 - m0
                        for vch in range(KC):
                            pv = ps.tile([128, 128], BF16, tag="ps", bufs=8)
                            nc.tensor.transpose(
                                pv[0:msz],
                                v_bf[vch][ihalf][:, c0 + m0:c0 + m1],
                                ident)
                            vt = wk.tile([128, 128], BF16,
                                         tag=f"vT{mi}_{vch}", bufs=2,
                                         name=f"vT{mi}_{vch}")
                            nc.vector.tensor_copy(out=vt[0:msz], in_=pv[0:msz])
                            vT2[mi][vch] = vt
                    for h in range(H):
                        ch, r0 = h // 2, (h % 2) * DK
                        qs = q_bf[ch][ihalf][r0:r0 + DK, c0:c0 + NTOK]
                        ks = k_bf[ch][ihalf][r0:r0 + DK, c0:c0 + NTOK]
                        vs = v_bf[ch][ihalf][r0:r0 + DK, c0:c0 + NTOK]
                        # scoresT chunks [m_sz, 197]; exp -> esT
                        esT = []
                        pssum = ps.tile([1, NTOK], F32, tag="ps", bufs=8)
                        for mi, (m0, m1) in enumerate(NSPLIT):
                            msz = m1 - m0
                            sT = ps.tile([128, NTOK], F32, tag="ps", bufs=8)
                            nc.tensor.matmul(sT[0:msz], ks[:, m0:m1], qs,
                                             start=True, stop=True)
                            e = wk.tile([128, NTOK], BF16, tag=f"esT{mi}",
                                        bufs=3, name=f"esT{mi}")
                            nc.scalar.activation(out=e[0:msz], in_=sT[0:msz],
                                                 func=AF.Exp, scale=SCALE)
                            esT.append(e)
                        for mi, (m0, m1) in enumerate(NSPLIT):
                            msz = m1 - m0
                            nc.tensor.matmul(pssum, ones_bf[0:msz],
                                             esT[mi][0:msz],
                                             start=(mi == 0), stop=(mi == 1))
                        rs = wk.tile([1, NTOK], F32, tag="rs", bufs=3)
                        nc.vector.reciprocal(rs, pssum[:, :])
                        rb = wk.tile([64, NTOK], F32, tag="rb", bufs=3)
                        nc.gpsimd.partition_broadcast(rb, rs)
                        pc = ps.tile([64, NTOK], F32, tag="ps", bufs=8)
                        for mi, (m0, m1) in enumerate(NSPLIT):
                            msz = m1 - m0
                            nc.tensor.matmul(pc, vT2[mi][ch][0:msz,
                                                            r0:r0 + DK],
                                             esT[mi][0:msz],
                                             start=(mi == 0), stop=(mi == 1))
                        nc.vector.tensor_mul(
                            ctx[ch][ihalf][r0:r0 + DK, c0:c0 + NTOK], pc, rb)

                # ---- O projection + residual ----
                wob_t = wk.tile([128, KC], F32, tag="wob", bufs=2)
                nc.sync.dma_start(out=wob_t,
                                  in_=wob[layer].rearrange("c p -> p c"))

                def wo_consumer(dc, half, p):
                    sl = tok[dc][half][:, :]
                    nc.vector.scalar_tensor_tensor(
                        out=sl, in0=p, scalar=wob_t[:, dc:dc + 1], in1=sl,
                        op0=OP.add, op1=OP.add)

                gemm_smallk(lambda kc: wow[layer, kc], KC,
                            lambda kc, half: ctx[kc][half][:, :],
                            wo_consumer)

                # ---- LN2 -> xhat2 ----
                xhat2 = wk.tile([128, KC, 2, HALF], F32R, tag="xhat", bufs=1)

                def ln2_dst(kc, half, tmp, bc):
                    nc.vector.tensor_mul(xhat2[:, kc, half, :], tmp, bc[:, 1, :])

                layernorm(lambda kc, half: tok[kc][half][:, :], ln2_dst)

                # ---- MLP1 + GELU -> hmlp (bf16, both halves) ----
                m1b_t = wk.tile([128, MC], F32, tag="m1b", bufs=2)
                nc.sync.dma_start(out=m1b_t,
                                  in_=m1b[layer].rearrange("c p -> p c"))

                def m1_consumer(dc, half, p):
                    nc.scalar.activation(
                        out=hmlp[:, dc, half * HALF:(half + 1) * HALF],
                        in_=p, func=AF.Gelu, bias=m1b_t[:, dc:dc + 1])

                gemm_smallk(lambda kc: m1w[layer, kc], MC,
                            lambda kc, half: xhat2[:, kc, half, :],
                            m1_consumer)

                # ---- MLP2 (bf16 weights) + residual ----
                m2b_t = wk.tile([128, KC], F32, tag="m2b", bufs=2)
                nc.sync.dma_start(out=m2b_t,
                                  in_=m2b[layer].rearrange("c p -> p c"))

                def m2_consumer(dc, half, p):
                    sl = tok[dc][half][:, :]
                    nc.vector.scalar_tensor_tensor(
                        out=sl, in0=p, scalar=m2b_t[:, dc:dc + 1], in1=sl,
                        op0=OP.add, op1=OP.add)

                gemm_smallk(lambda kc: m2w[layer, kc], KC,
                            lambda kc, half:
                            hmlp[:, kc, half * HALF:(half + 1) * HALF],
                            m2_consumer, G=2, kcn=MC, wdt=BF16)

            # ================= final head (cls tokens only) =================
            fnw_t = wk.tile([128, KC], F32, tag="fnw")
            fnb_t = wk.tile([128, KC], F32, tag="fnb")
            nc.sync.dma_start(out=fnw_t, in_=fnw.rearrange("c p -> p c"))
            nc.sync.dma_start(out=fnb_t, in_=fnb.rearrange("c p -> p c"))

            cls = wk.tile([128, KC, NIMG], F32, tag="cls")
            for c in range(KC):
                for h in range(2):
                    nc.vector.tensor_copy(out=cls[:, c, 2 * h:2 * h + 2],
                                          in_=tok[c][h][:, 0:2 * NTOK:NTOK])

            def head_ln(src, dst_fn):
                inv = 1.0 / D
                xb, xs = [], []
                for kc in range(KC):
                    b = wk.tile([128, NIMG], BF16, tag="hlnb", bufs=8)
                    nc.gpsimd.tensor_copy(out=b, in_=src(kc))
                    s = wk.tile([128, NIMG], BF16, tag="hlns", bufs=8)
                    nc.scalar.activation(out=s, in_=src(kc), func=AF.Square)
                    xb.append(b)
                    xs.append(s)
                psum = ps.tile([1, NIMG], F32, tag="ps", bufs=8)
                psq = ps.tile([1, NIMG], F32, tag="ps", bufs=8)
                for kc in range(KC):
                    nc.tensor.matmul(psum, ones_bf, xb[kc], start=(kc == 0),
                                     stop=(kc == KC - 1))
                for kc in range(KC):
                    nc.tensor.matmul(psq, ones_bf, xs[kc], start=(kc == 0),
                                     stop=(kc == KC - 1))
                st = wk.tile([1, 2, NIMG], F32, tag="hlnst", bufs=2)
                nc.scalar.mul(out=st[:, 0, :], in_=psum[:, :], mul=inv)
                m2 = wk.tile([1, NIMG], F32, tag="hlnm2", bufs=2)
                nc.vector.tensor_mul(m2, st[:, 0, :], st[:, 0, :])
                var = wk.tile([1, NIMG], F32, tag="hlnvar", bufs=2)
                nc.vector.scalar_tensor_tensor(out=var, in0=psq[:, :],
                                               scalar=inv, in1=m2,
                                               op0=OP.mult, op1=OP.subtract)
                nc.scalar.activation(out=st[:, 1, :], in_=var, func=AF.Sqrt,
                                     bias=eps_t)
                nc.vector.reciprocal(st[:, 1, :], st[:, 1, :])
                bc = wk.tile([128, 2, NIMG], F32, tag="hlnbc", bufs=2)
                nc.gpsimd.partition_broadcast(bc, st[0:1])
                for kc in range(KC):
                    tmp = wk.tile([128, NIMG], F32, tag="hlntmp", bufs=8)
                    nc.vector.tensor_sub(tmp, src(kc), bc[:, 0, :])
                    dst_fn(kc, tmp, bc)

            cls2 = wk.tile([128, KC, NIMG], F32, tag="cls2")

            def fn_dst(kc, tmp, bc):
                nc.vector.tensor_mul(tmp, tmp, bc[:, 1, :])
                nc.vector.tensor_scalar(
                    out=cls2[:, kc, :], in0=tmp, scalar1=fnw_t[:, kc:kc + 1],
                    scalar2=fnb_t[:, kc:kc + 1], op0=OP.mult, op1=OP.add)

            head_ln(lambda kc: cls[:, kc, :], fn_dst)

            clsr = wk.tile([128, KC, NIMG], F32R, tag="clsr")

            def hln_dst(kc, tmp, bc):
                nc.vector.tensor_mul(clsr[:, kc, :], tmp, bc[:, 1, :])

            head_ln(lambda kc: cls2[:, kc, :], hln_dst)

            # h1 GEMM + GELU (feature-major out [3072, NIMG])
            h1b_t = wk.tile([128, MC], F32, tag="h1b")
            nc.sync.dma_start(out=h1b_t, in_=h1b.rearrange("c p -> p c"))
            gh = wk.tile([128, MC, NIMG], F32R, tag="gh")
            for g0 in range(0, MC, 4):
                gsz = min(4, MC - g0)
                slabs = []
                for kc in range(KC):
                    wt = wp.tile([128, gsz * 128], F32R, tag="wslab", bufs=8)
                    nc.sync.dma_start(
                        out=wt,
                        in_=h1w[kc][:, g0 * 128:(g0 + gsz) * 128].bitcast(F32R))
                    slabs.append(wt)
                for dc in range(gsz):
                    p = ps.tile([128, NIMG], F32, tag="ps", bufs=8)
                    for kc in range(KC):
                        nc.tensor.matmul(p, slabs[kc][:, dc * 128:(dc + 1) * 128],
                                         clsr[:, kc, :],
                                         start=(kc == 0), stop=(kc == KC - 1))
                    nc.scalar.activation(out=gh[:, g0 + dc, :], in_=p,
                                         func=AF.Gelu,
                                         bias=h1b_t[:, g0 + dc:g0 + dc + 1])

            # h2 GEMM -> logits [2, NIMG]
            h2w_t = wk.tile([128, MC, 2], F32R, tag="h2w")
            nc.sync.dma_start(out=h2w_t,
                              in_=h2w.rearrange("c p n -> p c n").bitcast(F32R))
            h2b_t = wk.tile([2, 1], F32, tag="h2b")
            nc.sync.dma_start(out=h2b_t, in_=h2b[:, :])
            pl = ps.tile([2, NIMG], F32, tag="ps", bufs=8)
            for mc in range(MC):
                nc.tensor.matmul(pl, h2w_t[:, mc, :], gh[:, mc, :],
                                 start=(mc == 0), stop=(mc == MC - 1))
            logits = wk.tile([2, NIMG], F32, tag="logits")
            nc.vector.tensor_scalar_add(out=logits, in0=pl, scalar1=h2b_t)
            lap = logits[:, :]
            oap = out[:, :]
            dst = bass.AP(tensor=oap.tensor, offset=oap.offset,
                          ap=[[1, 2], [2, NIMG]])
            nc.sync.dma_start(out=dst, in_=lap)

    nc.finalize()
    return nc


def _prep_host(inputs):
    """Host-side weight/activation layout prep. Returns per-core input maps."""
    import ml_dtypes
    f = np.float32
    x = np.asarray(inputs["x"], f)
    B = x.shape[0]
    # patch matrix: d=(c,kh,kw) major -> [768, B, 196]
    xp = x.reshape(B, 3, 14, 16, 14, 16)
    pats = np.ascontiguousarray(xp.transpose(1, 3, 5, 0, 2, 4)).reshape(768, B, 196)

    conv_w = np.asarray(inputs["conv_w"], f)          # [D, 3, 16, 16]
    convw = np.ascontiguousarray(conv_w.reshape(D, 768).T).reshape(KC, 128, D)
    convb = np.asarray(inputs["conv_b"], f).reshape(KC, 128)

    cls_t = np.asarray(inputs["cls_token"], f).reshape(D)
    pos_e = np.asarray(inputs["pos_embed"], f).reshape(NTOK, D)
    clspos = (cls_t + pos_e[0]).reshape(KC, 128)
    pos = np.ascontiguousarray(pos_e.T).reshape(KC, 128, NTOK)

    ln1w = np.asarray(inputs["ln1_w"], f)
    ln1b = np.asarray(inputs["ln1_b"], f)
    ln2w = np.asarray(inputs["ln2_w"], f)
    ln2b = np.asarray(inputs["ln2_b"], f)
    wq, wk_, wv = (np.asarray(inputs[k], f) for k in ("wq", "wk", "wv"))
    wqkv = np.concatenate([wq, wk_, wv], axis=2)      # [L, D, 3D]
    qkvw = ln1w[:, :, None] * wqkv
    qkvb = np.einsum("ld,ldo->lo", ln1b, wqkv)
    qkvw = np.ascontiguousarray(qkvw).reshape(DEPTH, KC, 128, 3 * D)
    qkvb = np.ascontiguousarray(qkvb).reshape(DEPTH, 3 * KC, 128)

    wow = np.ascontiguousarray(np.asarray(inputs["wo_w"], f)).reshape(
        DEPTH, KC, 128, D)
    wob = np.asarray(inputs["wo_b"], f).reshape(DEPTH, KC, 128)

    m1 = np.asarray(inputs["mlp1_w"], f)              # [L, D, MLP]
    m1w = np.ascontiguousarray(ln2w[:, :, None] * m1).reshape(DEPTH, KC, 128, MLP)
    m1b = (np.asarray(inputs["mlp1_b"], f) +
           np.einsum("ld,ldo->lo", ln2b, m1)).reshape(DEPTH, MC, 128)
    m2w = np.ascontiguousarray(np.asarray(inputs["mlp2_w"], f)).reshape(
        DEPTH, MC, 128, D).astype(ml_dtypes.bfloat16)
    m2b = np.asarray(inputs["mlp2_b"], f).reshape(DEPTH, KC, 128)

    fnw = np.asarray(inputs["fnorm_w"], f).reshape(KC, 128)
    fnb = np.asarray(inputs["fnorm_b"], f).reshape(KC, 128)
    hlnw = np.asarray(inputs["hln_w"], f)
    hlnb = np.asarray(inputs["hln_b"], f)
    h1 = np.asarray(inputs["h1_w"], f)                # [D, MLP]
    h1w = np.ascontiguousarray(hlnw[:, None] * h1).reshape(KC, 128, MLP)
    h1b = (np.asarray(inputs["h1_b"], f) + hlnb @ h1).reshape(MC, 128)
    h2w = np.ascontiguousarray(np.asarray(inputs["h2_w"], f)).reshape(MC, 128, 2)
    h2b = np.asarray(inputs["h2_b"], f).reshape(2, 1)

    shared = dict(clspos=clspos, pos=pos, convw=convw, convb=convb,
                  qkvw=qkvw, qkvb=qkvb, wow=wow, wob=wob, m1w=m1w, m1b=m1b,
                  m2w=m2w, m2b=m2b, fnw=fnw, fnb=fnb, h1w=h1w, h1b=h1b,
                  h2w=h2w, h2b=h2b)
    per_core = []
    for c in range(NCORES):
        pc = np.ascontiguousarray(
            pats[:, c * NIMG:(c + 1) * NIMG, :]).reshape(KC, 128, NIMG * 196)
        per_core.append(dict(patches=pc, **shared))
    return per_core


def kernel(**inputs) -> np.ndarray:
    if "nc" not in _NC_CACHE:
        _NC_CACHE["nc"] = _build_nc()
    nc = _NC_CACHE["nc"]
    in_maps = _prep_host(inputs)
    trace = bool(int(os.environ.get("VIT_TRACE", "0")))
    res = run_bass_kernel_spmd(nc, in_maps, core_ids=list(range(NCORES)),
                               trace=trace)
    if trace:
        _NC_CACHE["last_exec_time_ns"] = res.exec_time_ns
        _NC_CACHE["last_result"] = res
    out = np.concatenate([res.results[c]["out"] for c in range(NCORES)], axis=0)
    return out.astype(np.float32)
